# revision 6
# baseline (speedup 1.0000x reference)
"""Trainium2 Bass kernel for multi-head causal attention.

Problem: q, k, v of shape [4096, 16, 64] (seq, heads, head_dim) fp32.
  out = softmax(causal(q @ k^T / 8)) @ v, reshaped to [4096, 1024].

Sharding: heads are split across 8 NeuronCores (2 heads per core).
Each core runs the same SPMD Bass program on its own 2 heads; the host
concatenates the per-core [4096, 128] outputs along the feature dim.

Per-core algorithm (flash-attention style, S^T orientation):
  - Load Q, K as bf16 (SWDGE cast DMA), transpose via the DMA XBAR
    (dma_start_transpose) into qT/kT [128=(h,d), 4096].
  - Load V per head into vplus [128, 32*65] bf16: each 128-row k-block
    gets 64 V columns plus a ones column (fused softmax denominator).
  - For each 512-wide q group G, per head h:
      mm1:  S^T[kj, qi] = kT_j^T.T @ qT_G into ps_h PSUM, 3 k-blocks per
            slot; the 4 diagonal blocks are PACKED (widths 512/384/256/128
            at offsets 0/512/1024/1280) so exp skips masked columns.
      exp:  one ScalarE activation per (group, head): Exp, scale=1/8,
            PSUM -> SBUF bf16.
      mask: diagonal group multiplied by one packed 0/1 causal mask (DVE).
      mm2:  V-STATIONARY: O^T[d|ones, qi] += vplus_j^T @ expS^T_j
            accumulated over ALL j in PSUM po_h [65, 512]. One N=512
            matmul per (j, head) instead of four M=128 ones.
  - Drain per G: DVE-copy po_h to bf16, DMA-XBAR transpose to [q, d]
    layout, reciprocal of the ones row, row-scale, DMA out.

mm2s are emitted one jgroup behind mm1/exp (software pipelining) so the
in-order PE queue always has ready work while ACT computes exp.

No distributed primitives are needed: sharding is purely host-side.
"""

import numpy as np

SEQ = 4096
NHEAD = 16
HDIM = 64
NCORES = 8
HPC = NHEAD // NCORES  # heads per core = 2
SCALE = 0.125

_NC_CACHE = {}
LAST_RESULT = {}

# Packed diagonal-group layout: per diag sub-block t, (psum offset, width).
DIAG_OFF = [0, 512, 1024, 1280]
DIAG_W = [512, 384, 256, 128]


def build_attention_nc(seq=SEQ, hpc=HPC, hdim=HDIM):
    """Build the SPMD Bass program for one core handling `hpc` heads."""
    import concourse.bass as bass
    import concourse.mybir as mybir
    import concourse.tile as tile

    f32 = mybir.dt.float32
    bf16 = mybir.dt.bfloat16
    Exp = mybir.ActivationFunctionType.Exp

    assert hpc == 2 and hdim == 64, "layout hardcoded for 2 heads x 64 dim"
    assert seq % 512 == 0
    nt = seq // 128   # number of 128-row seq tiles
    ng = seq // 512   # number of 512-wide q groups

    nc = bass.Bass()
    q = nc.dram_tensor("q", [seq, hpc, hdim], f32, kind="ExternalInput").ap()
    k = nc.dram_tensor("k", [seq, hpc, hdim], f32, kind="ExternalInput").ap()
    v = nc.dram_tensor("v", [seq, hpc, hdim], f32, kind="ExternalInput").ap()
    o = nc.dram_tensor("o", [seq, hpc * hdim], f32, kind="ExternalOutput").ap()

    with tile.TileContext(nc) as tc:
        with (
            tc.tile_pool(name="persist", bufs=1) as persist,
            tc.tile_pool(name="pexp", bufs=3) as pexp_pool,
            tc.tile_pool(name="outp", bufs=2) as out_pool,
            tc.tile_pool(name="small", bufs=4) as small_pool,
        ):
            # ---- persistent SBUF tensors ----------------------------------
            qT = persist.tile([128, seq], bf16, tag="qT")
            kT = persist.tile([128, seq], bf16, tag="kT")
            vplus = [
                persist.tile([128, nt * (hdim + 1)], bf16, tag=f"vplus{h}", name=f"vplus{h}")
                for h in range(hpc)
            ]
            # One packed multiplicative causal mask for the diagonal group:
            # local triangles (keep iff qi_local >= kj) at the 4 offsets,
            # zeros in the two pad gaps.
            maskp = persist.tile([128, 1536], bf16, tag="maskp")

            def build_masks():
                nc.vector.memset(maskp, 1.0)
                for t in range(4):
                    off, w = DIAG_OFF[t], DIAG_W[t]
                    nc.gpsimd.affine_select(
                        out=maskp[:, off : off + w],
                        in_=maskp[:, off : off + w],
                        compare_op=mybir.AluOpType.is_ge,
                        fill=0.0,
                        base=0,
                        pattern=[[1, w]],
                        channel_multiplier=-1,
                    )
                nc.vector.memset(maskp[:, 896:1024], 0.0)
                nc.vector.memset(maskp[:, 1408:1536], 0.0)

            # ---- V load: cast fp32->bf16 during DMA, ones pre-memset ------
            def load_v():
                for h in range(hpc):
                    nc.vector.memset(vplus[h], 1.0)
                    nc.gpsimd.dma_start(
                        out=vplus[h].rearrange("p (t x) -> p t x", x=hdim + 1)[:, :, 0:hdim],
                        in_=v[:, h, :].rearrange("(t p) d -> p t d", p=128),
                    )

            # ---- Q/K load + transpose (DMA XBAR; no PE/DVE involved) ------
            # One staging slot per chunk (bufs = #chunks): no slot reuse, so
            # cast DMAs have no anti-dependencies and stream back-to-back.
            # Priority order: chunk0 (k,q) first (gates the first mm1), then
            # masks + V (needed by the first diag exp / mm2), then the rest.
            chunk = min(8, nt)
            nchunks = 2 * (nt // chunk)
            with tc.tile_pool(name="ldstage", bufs=nchunks) as ld_pool:
                stages = []
                for cstart in range(0, nt, chunk):
                    for src, dstT in ((k, kT), (q, qT)):
                        src_r = src.rearrange("(t p) h d -> p t (h d)", p=128)
                        st = ld_pool.tile([128, chunk * 128], bf16, tag="ldstage")
                        nc.gpsimd.dma_start(
                            out=st.rearrange("p (t x) -> p t x", x=128),
                            in_=src_r[:, cstart : cstart + chunk, :],
                        )
                        stages.append((st, dstT, cstart))
                    if cstart == 0:
                        build_masks()
                        load_v()
                for st, dstT, cstart in stages:
                    nc.sync.dma_start_transpose(
                        out=dstT[
                            :, cstart * 128 : (cstart + chunk) * 128
                        ].rearrange("p (c j) -> p c j", j=128),
                        in_=st[:],
                    )

            # ---- main loop -------------------------------------------------
            with (
                tc.tile_pool(name="psum_s", bufs=1, space="PSUM") as ps_pool,
                tc.tile_pool(name="psum_o", bufs=1, space="PSUM") as po_pool,
            ):
                _main_loop(
                    nc, mybir, ng, hdim, ps_pool, po_pool, pexp_pool,
                    out_pool, small_pool, qT, kT, vplus, maskp, o, hpc, Exp,
                )
    _split_multi_waits(nc)
    return nc


def _split_multi_waits(nc):
    """Walrus's codegen accepts at most one sync-wait per instruction on
    this toolchain. Hoist extra waits into standalone single-wait NoOps on
    the same engine queue (same semantics: the sequencer stalls in order)."""
    import concourse.mybir as mybir

    nsplit = 0
    for blk in nc.m.functions[0].blocks:
        newl = []
        for ins in blk.instructions:
            si = getattr(ins, "sync_info", None)
            if si is not None and si.on_wait and len(si.on_wait) > 1:
                waits = list(si.on_wait)
                for w in waits[:-1]:
                    newl.append(
                        mybir.InstNoOp(
                            name=f"{ins.name}-wsplit{nsplit}",
                            sync_info=mybir.SyncInfo(on_wait=[w], on_update=[]),
                            bass_nofuse=True,
                            engine=ins.engine,
                            ins=[],
                            outs=[],
                        )
                    )
                    nsplit += 1
                ins.sync_info = mybir.SyncInfo(
                    on_wait=[waits[-1]], on_update=list(si.on_update or [])
                )
            newl.append(ins)
        blk.instructions = newl
    return nsplit


def _main_loop(nc, mybir, ng, hdim, ps_pool, po_pool, pexp_pool,
               out_pool, small_pool, qT, kT, vplus, maskp, o, hpc, Exp):
    SCALE = 0.125
    f32 = mybir.dt.float32
    bf16 = mybir.dt.bfloat16

    def emit_mm2s(st):
        """Deferred V-stationary P@V accumulation for one jgroup."""
        G, blocks, po, pes, njs, _ = st
        for h in range(hpc):
            for (j, off, w) in blocks:
                q0 = 512 - w  # valid q columns [q0:512] (0 for below-diag)
                nc.tensor.matmul(
                    po[h][0:hdim + 1, q0:512],
                    lhsT=vplus[h][:, j * 65 : j * 65 + hdim + 1],
                    rhs=pes[h][:, off : off + w],
                    start=(j == 0),
                    stop=(j == njs - 1),
                    skip_group_check=True,
                )

    def emit_drain(G, po):
        # O^T [65, 512] per head (PSUM fp32) -> bf16 SBUF, XBAR-transpose
        # to [q, d] chunks, normalize by the ones row, DMA out.
        oT = out_pool.tile([128, 1024], bf16, tag="oT", name="oT")
        for h in range(hpc):
            nc.vector.tensor_copy(
                oT[0 : hdim + 1, h * 512 : h * 512 + 512], po[h][:, :]
            )
        oTr = out_pool.tile([128, 1024], bf16, tag="oTr", name="oTr")
        nc.sync.dma_start_transpose(
            out=oTr.rearrange("p (c j) -> p c j", j=128), in_=oT[:]
        )
        rec = small_pool.tile([128, 8], f32, tag="rec", name="rec")
        oTr3 = oTr.rearrange("p (c j) -> p c j", j=128)
        nc.vector.reciprocal(rec, oTr3[:, :, hdim : hdim + 1])
        for cc in range(4):
            ob = out_pool.tile([128, hpc * hdim], f32, tag="ob", name="ob")
            for h in range(hpc):
                c = 4 * h + cc
                nc.vector.tensor_scalar_mul(
                    ob[:, h * hdim : (h + 1) * hdim],
                    oTr[:, c * 128 : c * 128 + hdim],
                    rec[:, c : c + 1],
                )
            blk = G * 4 + cc
            nc.sync.dma_start(out=o[blk * 128 : (blk + 1) * 128, :], in_=ob[:])

    pending = []  # deferred mm2 states (depth 2: PE stays 2 jgroups behind)
    for G in range(ng):
        njs = 4 * G + 4  # causal: k blocks 0 .. 4G+3
        po = [
            po_pool.tile([hdim + 1, 512], f32, tag=f"po{h}", name=f"po{h}")
            for h in range(hpc)
        ]
        # jgroups: below-diagonal full-width groups of <=3, then the packed
        # diagonal group (4 blocks at DIAG_OFF/DIAG_W).
        jgroups = []
        for s in range(0, 4 * G, 3):
            js = list(range(s, min(s + 3, 4 * G)))
            jgroups.append([(j, 512 * i, 512) for i, j in enumerate(js)])
        jgroups.append(
            [(4 * G + t, DIAG_OFF[t], DIAG_W[t]) for t in range(4)]
        )
        for gi, blocks in enumerate(jgroups):
            is_diag = gi == len(jgroups) - 1
            width = 1536 if is_diag else 512 * len(blocks)
            ps = [
                ps_pool.tile([128, 1536], f32, tag=f"ps{h}", name=f"ps{h}")
                for h in range(hpc)
            ]
            # mm1: S^T blocks, heads interleaved so LDWEIGHTS of the next
            # matmul (other 64-row group) overlaps the current stream.
            for (j, off, w) in blocks:
                t = j - 4 * G
                for h in range(hpc):
                    nc.tensor.matmul(
                        ps[h][:, off : off + w],
                        lhsT=kT[h * 64 : (h + 1) * 64, j * 128 : (j + 1) * 128],
                        rhs=qT[
                            h * 64 : (h + 1) * 64,
                            G * 512 + (512 - w) : (G + 1) * 512,
                        ],
                        start=not (is_diag and t == 3),
                        stop=True,
                        skip_group_check=True,
                        tile_position=(h * 64, 0),
                    )
            pes = []
            for h in range(hpc):
                pe = pexp_pool.tile(
                    [128, 1536], bf16, tag=f"pexp{h}", name=f"pexp{h}"
                )
                nc.scalar.activation(
                    out=pe[:, 0:width], in_=ps[h][:, 0:width], func=Exp,
                    scale=SCALE,
                )
                if is_diag:
                    nc.vector.tensor_mul(
                        pe[:, 0:1408], pe[:, 0:1408], maskp[:, 0:1408]
                    )
                pes.append(pe)
            pending.append((G, blocks, po, pes, njs, is_diag))
            if len(pending) > 2:
                st = pending.pop(0)
                emit_mm2s(st)
                if st[5]:  # was the last jgroup of its G
                    emit_drain(st[0], st[2])
    for st in pending:
        emit_mm2s(st)
        if st[5]:
            emit_drain(st[0], st[2])


def _ensure_ntff_hook():
    """The image's antenv package lacks axon_hooks; provide it so
    run_bass_kernel_spmd's trace path works (or degrades gracefully)."""
    import sys
    import types

    try:
        import antenv.axon_hooks  # noqa: F401

        return
    except ImportError:
        pass
    mod = types.ModuleType("antenv.axon_hooks")
    state = {"hook": None}
    mod.set_axon_ntff_profile_hook = lambda h: state.__setitem__("hook", h)
    mod.get_axon_ntff_profile_hook = lambda: state["hook"]
    try:
        from trn_agent_boot.trn_boot import _ntff_profile_via_ctypes

        state["hook"] = _ntff_profile_via_ctypes("/opt/axon/libaxon_pjrt.so")
    except Exception:
        state["hook"] = None
    sys.modules["antenv.axon_hooks"] = mod


def kernel(q, k, v):
    """Full-input entry point: q, k, v [4096, 16, 64] fp32 -> [4096, 1024]."""
    import sys

    if "/opt/trn_rl_repo" not in sys.path:
        sys.path.insert(0, "/opt/trn_rl_repo")
    _ensure_ntff_hook()
    from concourse.bass_utils import run_bass_kernel_spmd

    q = np.asarray(q, dtype=np.float32)
    k = np.asarray(k, dtype=np.float32)
    v = np.asarray(v, dtype=np.float32)
    seq, nhead, hdim = q.shape

    if "nc" not in _NC_CACHE:
        _NC_CACHE["nc"] = build_attention_nc(seq=seq, hpc=HPC, hdim=hdim)
    nc = _NC_CACHE["nc"]

    in_maps = []
    for c in range(NCORES):
        hs = slice(c * HPC, (c + 1) * HPC)
        in_maps.append(
            {
                "q": np.ascontiguousarray(q[:, hs, :]),
                "k": np.ascontiguousarray(k[:, hs, :]),
                "v": np.ascontiguousarray(v[:, hs, :]),
            }
        )
    res = run_bass_kernel_spmd(nc, in_maps, core_ids=list(range(NCORES)))
    LAST_RESULT["exec_time_ns"] = res.exec_time_ns
    try:
        iat = res.instructions_and_trace
        LAST_RESULT["trace_path"] = iat[1] if iat else None
    except Exception:
        LAST_RESULT["trace_path"] = None
    outs = [res.results[c]["o"] for c in range(NCORES)]
    return np.concatenate(outs, axis=1)


# revision 8
# speedup vs baseline: 1.1011x; 1.1011x over previous
"""Trainium2 Bass kernel for multi-head causal attention.

Problem: q, k, v of shape [4096, 16, 64] (seq, heads, head_dim) fp32.
  out = softmax(causal(q @ k^T / 8)) @ v, reshaped to [4096, 1024].

Sharding: heads are split across 8 NeuronCores (2 heads per core).
Each core runs the same SPMD Bass program on its own 2 heads; the host
concatenates the per-core [4096, 128] outputs along the feature dim.

Per-core algorithm (flash-attention style, S^T orientation):
  - Load Q, K as bf16 (SWDGE cast DMA), transpose via the DMA XBAR
    (dma_start_transpose) into qT/kT [128=(h,d), 4096].
  - Load V per head into vplus [128, 32*65] bf16: each 128-row k-block
    gets 64 V columns plus a ones column (fused softmax denominator).
  - For each 512-wide q group G, per head h:
      mm1:  S^T[kj, qi] = kT_j^T.T @ qT_G into ps_h PSUM, 3 k-blocks per
            slot; the 4 diagonal blocks are PACKED (widths 512/384/256/128
            at offsets 0/512/1024/1280) so exp skips masked columns.
      exp:  one ScalarE activation per (group, head): Exp, scale=1/8,
            PSUM -> SBUF bf16.
      mask: diagonal group multiplied by one packed 0/1 causal mask (DVE).
      mm2:  V-STATIONARY: O^T[d|ones, qi] += vplus_j^T @ expS^T_j
            accumulated over ALL j in PSUM po_h [65, 512]. One N=512
            matmul per (j, head) instead of four M=128 ones.
  - Drain per G: DVE-copy po_h to bf16, DMA-XBAR transpose to [q, d]
    layout, reciprocal of the ones row, row-scale, DMA out.

mm2s are emitted one jgroup behind mm1/exp (software pipelining) so the
in-order PE queue always has ready work while ACT computes exp.

No distributed primitives are needed: sharding is purely host-side.
"""

import numpy as np

SEQ = 4096
NHEAD = 16
HDIM = 64
NCORES = 8
HPC = NHEAD // NCORES  # heads per core = 2
SCALE = 0.125

_NC_CACHE = {}
LAST_RESULT = {}

# Packed diagonal-group layout: per diag sub-block t, (psum offset, width).
DIAG_OFF = [0, 512, 1024, 1280]
DIAG_W = [512, 384, 256, 128]


def build_attention_nc(seq=SEQ, hpc=HPC, hdim=HDIM):
    """Build the SPMD Bass program for one core handling `hpc` heads."""
    import concourse.bass as bass
    import concourse.mybir as mybir
    import concourse.tile as tile

    f32 = mybir.dt.float32
    bf16 = mybir.dt.bfloat16
    Exp = mybir.ActivationFunctionType.Exp

    assert hpc == 2 and hdim == 64, "layout hardcoded for 2 heads x 64 dim"
    assert seq % 512 == 0
    nt = seq // 128   # number of 128-row seq tiles
    ng = seq // 512   # number of 512-wide q groups

    nc = bass.Bass()
    q = nc.dram_tensor("q", [seq, hpc, hdim], f32, kind="ExternalInput").ap()
    k = nc.dram_tensor("k", [seq, hpc, hdim], f32, kind="ExternalInput").ap()
    v = nc.dram_tensor("v", [seq, hpc, hdim], f32, kind="ExternalInput").ap()
    o = nc.dram_tensor("o", [seq, hpc * hdim], f32, kind="ExternalOutput").ap()

    with tile.TileContext(nc) as tc:
        with (
            tc.tile_pool(name="persist", bufs=1) as persist,
            tc.tile_pool(name="pexp", bufs=3) as pexp_pool,
            tc.tile_pool(name="outp", bufs=2) as out_pool,
            tc.tile_pool(name="small", bufs=4) as small_pool,
        ):
            # ---- persistent SBUF tensors ----------------------------------
            qT = persist.tile([128, seq], bf16, tag="qT")
            kT = persist.tile([128, seq], bf16, tag="kT")
            vplus = [
                persist.tile([128, nt * (hdim + 1)], bf16, tag=f"vplus{h}", name=f"vplus{h}")
                for h in range(hpc)
            ]
            # One packed multiplicative causal mask for the diagonal group:
            # local triangles (keep iff qi_local >= kj) at the 4 offsets,
            # zeros in the two pad gaps.
            maskp = persist.tile([128, 1536], bf16, tag="maskp")

            def build_masks():
                nc.vector.memset(maskp, 1.0)
                for t in range(4):
                    off, w = DIAG_OFF[t], DIAG_W[t]
                    nc.gpsimd.affine_select(
                        out=maskp[:, off : off + w],
                        in_=maskp[:, off : off + w],
                        compare_op=mybir.AluOpType.is_ge,
                        fill=0.0,
                        base=0,
                        pattern=[[1, w]],
                        channel_multiplier=-1,
                    )
                nc.vector.memset(maskp[:, 896:1024], 0.0)
                nc.vector.memset(maskp[:, 1408:1536], 0.0)

            # ---- V load: cast fp32->bf16 during DMA, ones pre-memset ------
            def load_v():
                for h in range(hpc):
                    nc.vector.memset(vplus[h], 1.0)
                    nc.gpsimd.dma_start(
                        out=vplus[h].rearrange("p (t x) -> p t x", x=hdim + 1)[:, :, 0:hdim],
                        in_=v[:, h, :].rearrange("(t p) d -> p t d", p=128),
                    )

            # ---- Q/K load + transpose --------------------------------------
            # SWDGE cast DMAs are serial (single ucode queue), so order them
            # by criticality: chunk0 (tiles 0-7 of k then q, gates mm1 of
            # G0/G1), then the remaining 24 tiles as one big cast per tensor,
            # then V. Chunk0 is transposed on the PE (idle at startup, low
            # latency); the rest via the DMA XBAR (no engine cost, overlaps
            # the main loop's early groups).
            chunk = min(8, nt)
            rest = nt - chunk
            identity = persist.tile([128, 128], bf16, tag="identity")
            from concourse.masks import make_identity

            make_identity(nc, identity[:])
            with (
                tc.tile_pool(name="ldstage", bufs=1) as ld_pool,
                tc.tile_pool(name="psum_tr", bufs=4, space="PSUM") as tr_pool,
            ):
                st0 = {}
                for src, dstT, nm in ((k, kT, "k"), (q, qT, "q")):
                    src_r = src.rearrange("(t p) h d -> p t (h d)", p=128)
                    st = ld_pool.tile([128, chunk * 128], bf16, tag=f"ld0{nm}")
                    nc.gpsimd.dma_start(
                        out=st.rearrange("p (t x) -> p t x", x=128),
                        in_=src_r[:, 0:chunk, :],
                    )
                    st0[nm] = st
                build_masks()
                strest = {}
                for src, dstT, nm in ((k, kT, "k"), (q, qT, "q")):
                    src_r = src.rearrange("(t p) h d -> p t (h d)", p=128)
                    st = ld_pool.tile([128, rest * 128], bf16, tag=f"ldr{nm}")
                    nc.gpsimd.dma_start(
                        out=st.rearrange("p (t x) -> p t x", x=128),
                        in_=src_r[:, chunk:nt, :],
                    )
                    strest[nm] = st
                load_v()
                # chunk0: PE transpose + DVE copy into kT/qT
                for src, dstT, nm in ((k, kT, "k"), (q, qT, "q")):
                    st = st0[nm]
                    for tt in range(chunk):
                        ptr = tr_pool.tile([128, 128], bf16, tag="ptr", name="ptr")
                        nc.tensor.transpose(
                            ptr[:], st[:, tt * 128 : (tt + 1) * 128], identity[:]
                        )
                        nc.vector.tensor_copy(
                            dstT[:, tt * 128 : (tt + 1) * 128], ptr[:]
                        )
                # rest: one XBAR transpose per tensor
                for src, dstT, nm in ((k, kT, "k"), (q, qT, "q")):
                    nc.sync.dma_start_transpose(
                        out=dstT[:, chunk * 128 :].rearrange(
                            "p (c j) -> p c j", j=128
                        ),
                        in_=strest[nm][:],
                    )

            # ---- main loop -------------------------------------------------
            with (
                tc.tile_pool(name="psum_s", bufs=1, space="PSUM") as ps_pool,
                tc.tile_pool(name="psum_o", bufs=1, space="PSUM") as po_pool,
            ):
                _main_loop(
                    nc, mybir, ng, hdim, ps_pool, po_pool, pexp_pool,
                    out_pool, small_pool, qT, kT, vplus, maskp, o, hpc, Exp,
                )
    _split_multi_waits(nc)
    return nc


def _split_multi_waits(nc):
    """Walrus's codegen accepts at most one sync-wait per instruction on
    this toolchain. Hoist extra waits into standalone single-wait NoOps on
    the same engine queue (same semantics: the sequencer stalls in order)."""
    import concourse.mybir as mybir

    nsplit = 0
    for blk in nc.m.functions[0].blocks:
        newl = []
        for ins in blk.instructions:
            si = getattr(ins, "sync_info", None)
            if si is not None and si.on_wait and len(si.on_wait) > 1:
                waits = list(si.on_wait)
                for w in waits[:-1]:
                    newl.append(
                        mybir.InstNoOp(
                            name=f"{ins.name}-wsplit{nsplit}",
                            sync_info=mybir.SyncInfo(on_wait=[w], on_update=[]),
                            bass_nofuse=True,
                            engine=ins.engine,
                            ins=[],
                            outs=[],
                        )
                    )
                    nsplit += 1
                ins.sync_info = mybir.SyncInfo(
                    on_wait=[waits[-1]], on_update=list(si.on_update or [])
                )
            newl.append(ins)
        blk.instructions = newl
    return nsplit


def _main_loop(nc, mybir, ng, hdim, ps_pool, po_pool, pexp_pool,
               out_pool, small_pool, qT, kT, vplus, maskp, o, hpc, Exp):
    SCALE = 0.125
    f32 = mybir.dt.float32
    bf16 = mybir.dt.bfloat16

    def emit_mm2s(st):
        """Deferred V-stationary P@V accumulation for one jgroup."""
        G, blocks, po, pes, njs, _ = st
        for h in range(hpc):
            for (j, off, w) in blocks:
                q0 = 512 - w  # valid q columns [q0:512] (0 for below-diag)
                nc.tensor.matmul(
                    po[h][0:hdim + 1, q0:512],
                    lhsT=vplus[h][:, j * 65 : j * 65 + hdim + 1],
                    rhs=pes[h][:, off : off + w],
                    start=(j == 0),
                    stop=(j == njs - 1),
                    skip_group_check=True,
                )

    def emit_drain(G, po):
        # O^T [65, 512] per head (PSUM fp32) -> bf16 SBUF, XBAR-transpose
        # to [q, d] chunks, normalize by the ones row, DMA out.
        oT = out_pool.tile([128, 1024], bf16, tag="oT", name="oT")
        for h in range(hpc):
            nc.vector.tensor_copy(
                oT[0 : hdim + 1, h * 512 : h * 512 + 512], po[h][:, :]
            )
        oTr = out_pool.tile([128, 1024], bf16, tag="oTr", name="oTr")
        nc.sync.dma_start_transpose(
            out=oTr.rearrange("p (c j) -> p c j", j=128), in_=oT[:]
        )
        rec = small_pool.tile([128, 8], f32, tag="rec", name="rec")
        oTr3 = oTr.rearrange("p (c j) -> p c j", j=128)
        nc.vector.reciprocal(rec, oTr3[:, :, hdim : hdim + 1])
        for cc in range(4):
            ob = out_pool.tile([128, hpc * hdim], f32, tag="ob", name="ob")
            for h in range(hpc):
                c = 4 * h + cc
                nc.vector.tensor_scalar_mul(
                    ob[:, h * hdim : (h + 1) * hdim],
                    oTr[:, c * 128 : c * 128 + hdim],
                    rec[:, c : c + 1],
                )
            blk = G * 4 + cc
            nc.sync.dma_start(out=o[blk * 128 : (blk + 1) * 128, :], in_=ob[:])

    pending = []  # deferred mm2 states (depth 2: PE stays 2 jgroups behind)
    for G in range(ng):
        njs = 4 * G + 4  # causal: k blocks 0 .. 4G+3
        po = [
            po_pool.tile([hdim + 1, 512], f32, tag=f"po{h}", name=f"po{h}")
            for h in range(hpc)
        ]
        # jgroups: below-diagonal full-width groups of <=3, then the packed
        # diagonal group (4 blocks at DIAG_OFF/DIAG_W).
        jgroups = []
        for s in range(0, 4 * G, 3):
            js = list(range(s, min(s + 3, 4 * G)))
            jgroups.append([(j, 512 * i, 512) for i, j in enumerate(js)])
        jgroups.append(
            [(4 * G + t, DIAG_OFF[t], DIAG_W[t]) for t in range(4)]
        )
        for gi, blocks in enumerate(jgroups):
            is_diag = gi == len(jgroups) - 1
            width = 1536 if is_diag else 512 * len(blocks)
            ps = [
                ps_pool.tile([128, 1536], f32, tag=f"ps{h}", name=f"ps{h}")
                for h in range(hpc)
            ]
            # mm1: S^T blocks, heads interleaved so LDWEIGHTS of the next
            # matmul (other 64-row group) overlaps the current stream.
            for (j, off, w) in blocks:
                t = j - 4 * G
                for h in range(hpc):
                    nc.tensor.matmul(
                        ps[h][:, off : off + w],
                        lhsT=kT[h * 64 : (h + 1) * 64, j * 128 : (j + 1) * 128],
                        rhs=qT[
                            h * 64 : (h + 1) * 64,
                            G * 512 + (512 - w) : (G + 1) * 512,
                        ],
                        start=not (is_diag and t == 3),
                        stop=True,
                        skip_group_check=True,
                        tile_position=(h * 64, 0),
                    )
            pes = []
            for h in range(hpc):
                pe = pexp_pool.tile(
                    [128, 1536], bf16, tag=f"pexp{h}", name=f"pexp{h}"
                )
                nc.scalar.activation(
                    out=pe[:, 0:width], in_=ps[h][:, 0:width], func=Exp,
                    scale=SCALE,
                )
                if is_diag:
                    nc.vector.tensor_mul(
                        pe[:, 0:1408], pe[:, 0:1408], maskp[:, 0:1408]
                    )
                pes.append(pe)
            pending.append((G, blocks, po, pes, njs, is_diag))
            if len(pending) > 1:
                st = pending.pop(0)
                emit_mm2s(st)
                if st[5]:  # was the last jgroup of its G
                    emit_drain(st[0], st[2])
    for st in pending:
        emit_mm2s(st)
        if st[5]:
            emit_drain(st[0], st[2])


def _ensure_ntff_hook():
    """The image's antenv package lacks axon_hooks; provide it so
    run_bass_kernel_spmd's trace path works (or degrades gracefully)."""
    import sys
    import types

    try:
        import antenv.axon_hooks  # noqa: F401

        return
    except ImportError:
        pass
    mod = types.ModuleType("antenv.axon_hooks")
    state = {"hook": None}
    mod.set_axon_ntff_profile_hook = lambda h: state.__setitem__("hook", h)
    mod.get_axon_ntff_profile_hook = lambda: state["hook"]
    try:
        from trn_agent_boot.trn_boot import _ntff_profile_via_ctypes

        state["hook"] = _ntff_profile_via_ctypes("/opt/axon/libaxon_pjrt.so")
    except Exception:
        state["hook"] = None
    sys.modules["antenv.axon_hooks"] = mod


def kernel(q, k, v):
    """Full-input entry point: q, k, v [4096, 16, 64] fp32 -> [4096, 1024]."""
    import sys

    if "/opt/trn_rl_repo" not in sys.path:
        sys.path.insert(0, "/opt/trn_rl_repo")
    _ensure_ntff_hook()
    from concourse.bass_utils import run_bass_kernel_spmd

    q = np.asarray(q, dtype=np.float32)
    k = np.asarray(k, dtype=np.float32)
    v = np.asarray(v, dtype=np.float32)
    seq, nhead, hdim = q.shape

    if "nc" not in _NC_CACHE:
        _NC_CACHE["nc"] = build_attention_nc(seq=seq, hpc=HPC, hdim=hdim)
    nc = _NC_CACHE["nc"]

    in_maps = []
    for c in range(NCORES):
        hs = slice(c * HPC, (c + 1) * HPC)
        in_maps.append(
            {
                "q": np.ascontiguousarray(q[:, hs, :]),
                "k": np.ascontiguousarray(k[:, hs, :]),
                "v": np.ascontiguousarray(v[:, hs, :]),
            }
        )
    res = run_bass_kernel_spmd(nc, in_maps, core_ids=list(range(NCORES)))
    LAST_RESULT["exec_time_ns"] = res.exec_time_ns
    try:
        iat = res.instructions_and_trace
        LAST_RESULT["trace_path"] = iat[1] if iat else None
    except Exception:
        LAST_RESULT["trace_path"] = None
    outs = [res.results[c]["o"] for c in range(NCORES)]
    return np.concatenate(outs, axis=1)


# revision 15
# speedup vs baseline: 1.1148x; 1.0125x over previous
"""Trainium2 Bass kernel for multi-head causal attention.

Problem: q, k, v of shape [4096, 16, 64] (seq, heads, head_dim) fp32.
  out = softmax(causal(q @ k^T / 8)) @ v, reshaped to [4096, 1024].

Sharding: heads are split across 8 NeuronCores (2 heads per core).
Each core runs the same SPMD Bass program on its own 2 heads; the host
concatenates the per-core [4096, 128] outputs along the feature dim.

Per-core algorithm (flash-attention style, S^T orientation):
  - Load Q, K as bf16 (SWDGE cast DMA), transpose via the DMA XBAR
    (dma_start_transpose) into qT/kT [128=(h,d), 4096].
  - Load V per head into vplus [128, 32*65] bf16: each 128-row k-block
    gets 64 V columns plus a ones column (fused softmax denominator).
  - For each 512-wide q group G, per head h:
      mm1:  S^T[kj, qi] = kT_j^T.T @ qT_G into ps_h PSUM, 3 k-blocks per
            slot; the 4 diagonal blocks are PACKED (widths 512/384/256/128
            at offsets 0/512/1024/1280) so exp skips masked columns.
      exp:  one ScalarE activation per (group, head): Exp, scale=1/8,
            PSUM -> SBUF bf16.
      mask: diagonal group multiplied by one packed 0/1 causal mask (DVE).
      mm2:  V-STATIONARY: O^T[d|ones, qi] += vplus_j^T @ expS^T_j
            accumulated over ALL j in PSUM po_h [65, 512]. One N=512
            matmul per (j, head) instead of four M=128 ones.
  - Drain per G: DVE-copy po_h to bf16, DMA-XBAR transpose to [q, d]
    layout, reciprocal of the ones row, row-scale, DMA out.

mm2s are emitted one jgroup behind mm1/exp (software pipelining) so the
in-order PE queue always has ready work while ACT computes exp.

No distributed primitives are needed: sharding is purely host-side.
"""

import numpy as np

SEQ = 4096
NHEAD = 16
HDIM = 64
NCORES = 8
HPC = NHEAD // NCORES  # heads per core = 2
SCALE = 0.125

_NC_CACHE = {}
LAST_RESULT = {}

# Packed diagonal-group layout: per diag sub-block t, (psum offset, width).
DIAG_OFF = [0, 512, 1024, 1280]
DIAG_W = [512, 384, 256, 128]


def build_attention_nc(seq=SEQ, hpc=HPC, hdim=HDIM):
    """Build the SPMD Bass program for one core handling `hpc` heads."""
    import concourse.bass as bass
    import concourse.mybir as mybir
    import concourse.tile as tile

    f32 = mybir.dt.float32
    bf16 = mybir.dt.bfloat16
    Exp = mybir.ActivationFunctionType.Exp

    assert hpc == 2 and hdim == 64, "layout hardcoded for 2 heads x 64 dim"
    assert seq % 512 == 0
    nt = seq // 128   # number of 128-row seq tiles
    ng = seq // 512   # number of 512-wide q groups

    nc = bass.Bass()
    q = nc.dram_tensor("q", [seq, hpc, hdim], f32, kind="ExternalInput").ap()
    k = nc.dram_tensor("k", [seq, hpc, hdim], f32, kind="ExternalInput").ap()
    v = nc.dram_tensor("v", [seq, hpc, hdim], f32, kind="ExternalInput").ap()
    o = nc.dram_tensor("o", [seq, hpc * hdim], f32, kind="ExternalOutput").ap()

    with tile.TileContext(nc) as tc:
        with (
            tc.tile_pool(name="persist", bufs=1) as persist,
            tc.tile_pool(name="pexp", bufs=3) as pexp_pool,
            tc.tile_pool(name="outp", bufs=2) as out_pool,
            tc.tile_pool(name="small", bufs=4) as small_pool,
        ):
            # ---- persistent SBUF tensors ----------------------------------
            qT = persist.tile([128, seq], bf16, tag="qT")
            kT = persist.tile([128, seq], bf16, tag="kT")
            vplus = [
                persist.tile([128, nt * (hdim + 1)], bf16, tag=f"vplus{h}", name=f"vplus{h}")
                for h in range(hpc)
            ]
            # One packed multiplicative causal mask for the diagonal group:
            # local triangles (keep iff qi_local >= kj) at the 4 offsets,
            # zeros in the two pad gaps.
            maskp = persist.tile([128, 1536], bf16, tag="maskp")

            def build_masks():
                nc.vector.memset(maskp, 1.0)
                for t in range(4):
                    off, w = DIAG_OFF[t], DIAG_W[t]
                    nc.gpsimd.affine_select(
                        out=maskp[:, off : off + w],
                        in_=maskp[:, off : off + w],
                        compare_op=mybir.AluOpType.is_ge,
                        fill=0.0,
                        base=0,
                        pattern=[[1, w]],
                        channel_multiplier=-1,
                    )
                nc.vector.memset(maskp[:, 896:1024], 0.0)
                nc.vector.memset(maskp[:, 1408:1536], 0.0)

            # ---- V load: cast fp32->bf16 during DMA, ones pre-memset ------
            def load_v():
                for h in range(hpc):
                    nc.vector.memset(vplus[h], 1.0)
                    nc.gpsimd.dma_start(
                        out=vplus[h].rearrange("p (t x) -> p t x", x=hdim + 1)[:, :, 0:hdim],
                        in_=v[:, h, :].rearrange("(t p) d -> p t d", p=128),
                    )

            # ---- Q/K load + transpose --------------------------------------
            # SWDGE cast DMAs are serial (single ucode queue), so order them
            # by criticality: chunk0 (tiles 0-7 of k then q, gates mm1 of
            # G0/G1), then the remaining 24 tiles as one big cast per tensor,
            # then V. Chunk0 is transposed on the PE (idle at startup, low
            # latency); the rest via the DMA XBAR (no engine cost, overlaps
            # the main loop's early groups).
            chunk = min(8, nt)
            rest = nt - chunk
            identity = persist.tile([128, 128], bf16, tag="identity")
            from concourse.masks import make_identity

            make_identity(nc, identity[:])
            with (
                tc.tile_pool(name="ldstage", bufs=1) as ld_pool,
                tc.tile_pool(name="psum_tr", bufs=4, space="PSUM") as tr_pool,
            ):
                srcs = {"k": (k, kT), "q": (q, qT)}

                def cast_load(nm, t0, t1):
                    src, _ = srcs[nm]
                    src_r = src.rearrange("(t p) h d -> p t (h d)", p=128)
                    st = ld_pool.tile(
                        [128, (t1 - t0) * 128], bf16, tag=f"ld{nm}{t0}"
                    )
                    nc.gpsimd.dma_start(
                        out=st.rearrange("p (t x) -> p t x", x=128),
                        in_=src_r[:, t0:t1, :],
                    )
                    return st

                def xbar_tr(nm, st, t0, ntile):
                    _, dstT = srcs[nm]
                    nc.sync.dma_start_transpose(
                        out=dstT[
                            :, t0 * 128 : (t0 + ntile) * 128
                        ].rearrange("p (c j) -> p c j", j=128),
                        in_=st[:],
                    )

                # chunk0 (tiles 0-7) casts first, then c1 (8-15, gates G2/G3),
                # then V, then c23 (16-31, not needed until ~G4).
                st0 = {nm: cast_load(nm, 0, chunk) for nm in ("k", "q")}
                build_masks()
                st1 = {nm: cast_load(nm, chunk, 2 * chunk) for nm in ("k", "q")}
                for nm in ("k", "q"):
                    xbar_tr(nm, st1[nm], chunk, chunk)
                load_v()
                st2 = {nm: cast_load(nm, 2 * chunk, nt) for nm in ("k", "q")}
                for nm in ("k", "q"):
                    xbar_tr(nm, st2[nm], 2 * chunk, nt - 2 * chunk)
                # chunk0: PE transpose + DVE copy into kT/qT, interleaved in
                # 4-tile pieces so mm1 of G0 (tiles 0-3) unblocks earliest.
                for piece in range(chunk // 4):
                    for nm in ("k", "q"):
                        _, dstT = srcs[nm]
                        st = st0[nm]
                        for tt in range(piece * 4, piece * 4 + 4):
                            ptr = tr_pool.tile(
                                [128, 128], bf16, tag="ptr", name="ptr"
                            )
                            nc.tensor.transpose(
                                ptr[:], st[:, tt * 128 : (tt + 1) * 128],
                                identity[:],
                            )
                            nc.vector.tensor_copy(
                                dstT[:, tt * 128 : (tt + 1) * 128], ptr[:]
                            )

            # ---- main loop -------------------------------------------------
            with (
                tc.tile_pool(name="psum_s", bufs=1, space="PSUM") as ps_pool,
                tc.tile_pool(name="psum_o", bufs=1, space="PSUM") as po_pool,
            ):
                _main_loop(
                    nc, mybir, ng, hdim, ps_pool, po_pool, pexp_pool,
                    out_pool, small_pool, qT, kT, vplus, maskp, o, hpc, Exp,
                )
    _split_multi_waits(nc)
    return nc


def _split_multi_waits(nc):
    """Walrus's codegen accepts at most one sync-wait per instruction on
    this toolchain. Hoist extra waits into standalone single-wait NoOps on
    the same engine queue (same semantics: the sequencer stalls in order)."""
    import concourse.mybir as mybir

    nsplit = 0
    for blk in nc.m.functions[0].blocks:
        newl = []
        for ins in blk.instructions:
            si = getattr(ins, "sync_info", None)
            if si is not None and si.on_wait and len(si.on_wait) > 1:
                waits = list(si.on_wait)
                for w in waits[:-1]:
                    newl.append(
                        mybir.InstNoOp(
                            name=f"{ins.name}-wsplit{nsplit}",
                            sync_info=mybir.SyncInfo(on_wait=[w], on_update=[]),
                            bass_nofuse=True,
                            engine=ins.engine,
                            ins=[],
                            outs=[],
                        )
                    )
                    nsplit += 1
                ins.sync_info = mybir.SyncInfo(
                    on_wait=[waits[-1]], on_update=list(si.on_update or [])
                )
            newl.append(ins)
        blk.instructions = newl
    return nsplit


def _main_loop(nc, mybir, ng, hdim, ps_pool, po_pool, pexp_pool,
               out_pool, small_pool, qT, kT, vplus, maskp, o, hpc, Exp):
    SCALE = 0.125
    f32 = mybir.dt.float32
    bf16 = mybir.dt.bfloat16

    def emit_mm2s(st, drain):
        """Deferred V-stationary P@V accumulation for one jgroup. When
        `drain` is set this is the last jgroup of its G: each head's O^T is
        drained right after that head's final mm2 so the copy/transpose/
        normalize chain overlaps the other head's matmuls."""
        G, blocks, po, pes, njs, _ = st
        for h in range(hpc):
            for (j, off, w) in blocks:
                q0 = 512 - w  # valid q columns [q0:512] (0 for below-diag)
                nc.tensor.matmul(
                    po[h][0:hdim + 1, q0:512],
                    lhsT=vplus[h][:, j * 65 : j * 65 + hdim + 1],
                    rhs=pes[h][:, off : off + w],
                    start=(j == 0),
                    stop=(j == njs - 1),
                    skip_group_check=True,
                )
            if drain:
                emit_drain_head(G, po, h)
        if drain:
            emit_drain_finish(G)

    def emit_drain_head(G, po, h):
        # O^T [65, 512] (PSUM fp32) -> bf16 SBUF, XBAR-transpose to [q, d].
        oT = out_pool.tile([128, 512], bf16, tag=f"oT{h}", name=f"oT{h}")
        nc.vector.tensor_copy(oT[0 : hdim + 1, :], po[h][:, :])
        oTr = drain_state[h] = out_pool.tile(
            [128, 512], bf16, tag=f"oTr{h}", name=f"oTr{h}"
        )
        nc.sync.dma_start_transpose(
            out=oTr.rearrange("p (c j) -> p c j", j=128), in_=oT[:]
        )

    def emit_drain_finish(G):
        # reciprocal of the ones rows, scale, interleave heads, DMA out.
        for cc in range(4):
            ob = out_pool.tile([128, hpc * hdim], f32, tag="ob", name="ob")
            for h in range(hpc):
                rec = small_pool.tile([128, 1], f32, tag="rec", name="rec")
                nc.vector.reciprocal(
                    rec,
                    drain_state[h][:, cc * 128 + hdim : cc * 128 + hdim + 1],
                )
                nc.vector.tensor_scalar_mul(
                    ob[:, h * hdim : (h + 1) * hdim],
                    drain_state[h][:, cc * 128 : cc * 128 + hdim],
                    rec,
                )
            blk = G * 4 + cc
            nc.sync.dma_start(out=o[blk * 128 : (blk + 1) * 128, :], in_=ob[:])

    drain_state = {}

    pending = []  # deferred mm2 states (depth 2: PE stays 2 jgroups behind)
    for G in range(ng):
        njs = 4 * G + 4  # causal: k blocks 0 .. 4G+3
        po = [
            po_pool.tile([hdim + 1, 512], f32, tag=f"po{h}", name=f"po{h}")
            for h in range(hpc)
        ]
        # jgroups: below-diagonal full-width groups of <=3, then the packed
        # diagonal group (4 blocks at DIAG_OFF/DIAG_W).
        jgroups = []
        for s in range(0, 4 * G, 3):
            js = list(range(s, min(s + 3, 4 * G)))
            jgroups.append([(j, 512 * i, 512) for i, j in enumerate(js)])
        jgroups.append(
            [(4 * G + t, DIAG_OFF[t], DIAG_W[t]) for t in range(4)]
        )
        for gi, blocks in enumerate(jgroups):
            is_diag = gi == len(jgroups) - 1
            width = 1536 if is_diag else 512 * len(blocks)
            ps = [
                ps_pool.tile([128, 1536], f32, tag=f"ps{h}", name=f"ps{h}")
                for h in range(hpc)
            ]
            # mm1: S^T blocks, heads interleaved so LDWEIGHTS of the next
            # matmul (other 64-row group) overlaps the current stream.
            for (j, off, w) in blocks:
                t = j - 4 * G
                for h in range(hpc):
                    nc.tensor.matmul(
                        ps[h][:, off : off + w],
                        lhsT=kT[h * 64 : (h + 1) * 64, j * 128 : (j + 1) * 128],
                        rhs=qT[
                            h * 64 : (h + 1) * 64,
                            G * 512 + (512 - w) : (G + 1) * 512,
                        ],
                        start=not (is_diag and t == 3),
                        stop=True,
                        skip_group_check=True,
                        tile_position=(h * 64, 0),
                    )
            ew = 1408 if is_diag else width  # [1408:1536] is never consumed
            pes = []
            for h in range(hpc):
                pe = pexp_pool.tile(
                    [128, 1536], bf16, tag=f"pexp{h}", name=f"pexp{h}"
                )
                nc.scalar.activation(
                    out=pe[:, 0:ew], in_=ps[h][:, 0:ew], func=Exp,
                    scale=SCALE,
                )
                if is_diag:
                    nc.vector.tensor_mul(
                        pe[:, 0:1408], pe[:, 0:1408], maskp[:, 0:1408]
                    )
                pes.append(pe)
            pending.append((G, blocks, po, pes, njs, is_diag))
            if len(pending) > 1:
                st = pending.pop(0)
                emit_mm2s(st, st[5])
    for st in pending:
        emit_mm2s(st, st[5])


def _ensure_ntff_hook():
    """The image's antenv package lacks axon_hooks; provide it so
    run_bass_kernel_spmd's trace path works (or degrades gracefully)."""
    import sys
    import types

    try:
        import antenv.axon_hooks  # noqa: F401

        return
    except ImportError:
        pass
    mod = types.ModuleType("antenv.axon_hooks")
    state = {"hook": None}
    mod.set_axon_ntff_profile_hook = lambda h: state.__setitem__("hook", h)
    mod.get_axon_ntff_profile_hook = lambda: state["hook"]
    try:
        from trn_agent_boot.trn_boot import _ntff_profile_via_ctypes

        state["hook"] = _ntff_profile_via_ctypes("/opt/axon/libaxon_pjrt.so")
    except Exception:
        state["hook"] = None
    sys.modules["antenv.axon_hooks"] = mod


def kernel(q, k, v):
    """Full-input entry point: q, k, v [4096, 16, 64] fp32 -> [4096, 1024]."""
    import sys

    if "/opt/trn_rl_repo" not in sys.path:
        sys.path.insert(0, "/opt/trn_rl_repo")
    _ensure_ntff_hook()
    from concourse.bass_utils import run_bass_kernel_spmd

    q = np.asarray(q, dtype=np.float32)
    k = np.asarray(k, dtype=np.float32)
    v = np.asarray(v, dtype=np.float32)
    seq, nhead, hdim = q.shape

    if "nc" not in _NC_CACHE:
        _NC_CACHE["nc"] = build_attention_nc(seq=seq, hpc=HPC, hdim=hdim)
    nc = _NC_CACHE["nc"]

    in_maps = []
    for c in range(NCORES):
        hs = slice(c * HPC, (c + 1) * HPC)
        in_maps.append(
            {
                "q": np.ascontiguousarray(q[:, hs, :]),
                "k": np.ascontiguousarray(k[:, hs, :]),
                "v": np.ascontiguousarray(v[:, hs, :]),
            }
        )
    res = run_bass_kernel_spmd(nc, in_maps, core_ids=list(range(NCORES)))
    LAST_RESULT["exec_time_ns"] = res.exec_time_ns
    try:
        iat = res.instructions_and_trace
        LAST_RESULT["trace_path"] = iat[1] if iat else None
    except Exception:
        LAST_RESULT["trace_path"] = None
    outs = [res.results[c]["o"] for c in range(NCORES)]
    return np.concatenate(outs, axis=1)


# revision 17
# speedup vs baseline: 1.1222x; 1.0066x over previous
"""Trainium2 Bass kernel for multi-head causal attention.

Problem: q, k, v of shape [4096, 16, 64] (seq, heads, head_dim) fp32.
  out = softmax(causal(q @ k^T / 8)) @ v, reshaped to [4096, 1024].

Sharding: heads are split across 8 NeuronCores (2 heads per core).
Each core runs the same SPMD Bass program on its own 2 heads; the host
concatenates the per-core [4096, 128] outputs along the feature dim.

Per-core algorithm (flash-attention style, S^T orientation):
  - Load Q, K as bf16 (SWDGE cast DMA), transpose via the DMA XBAR
    (dma_start_transpose) into qT/kT [128=(h,d), 4096].
  - Load V per head into vplus [128, 32*65] bf16: each 128-row k-block
    gets 64 V columns plus a ones column (fused softmax denominator).
  - For each 512-wide q group G, per head h:
      mm1:  S^T[kj, qi] = kT_j^T.T @ qT_G into ps_h PSUM, 3 k-blocks per
            slot; the 4 diagonal blocks are PACKED (widths 512/384/256/128
            at offsets 0/512/1024/1280) so exp skips masked columns.
      exp:  one ScalarE activation per (group, head): Exp, scale=1/8,
            PSUM -> SBUF bf16.
      mask: diagonal group multiplied by one packed 0/1 causal mask (DVE).
      mm2:  V-STATIONARY: O^T[d|ones, qi] += vplus_j^T @ expS^T_j
            accumulated over ALL j in PSUM po_h [65, 512]. One N=512
            matmul per (j, head) instead of four M=128 ones.
  - Drain per G: DVE-copy po_h to bf16, DMA-XBAR transpose to [q, d]
    layout, reciprocal of the ones row, row-scale, DMA out.

mm2s are emitted one jgroup behind mm1/exp (software pipelining) so the
in-order PE queue always has ready work while ACT computes exp.

No distributed primitives are needed: sharding is purely host-side.
"""

import numpy as np

SEQ = 4096
NHEAD = 16
HDIM = 64
NCORES = 8
HPC = NHEAD // NCORES  # heads per core = 2
SCALE = 0.125

_NC_CACHE = {}
LAST_RESULT = {}

# Packed diagonal-group layout: per diag sub-block t, (psum offset, width).
DIAG_OFF = [0, 512, 1024, 1280]
DIAG_W = [512, 384, 256, 128]


def build_attention_nc(seq=SEQ, hpc=HPC, hdim=HDIM):
    """Build the SPMD Bass program for one core handling `hpc` heads."""
    import concourse.bass as bass
    import concourse.mybir as mybir
    import concourse.tile as tile

    f32 = mybir.dt.float32
    bf16 = mybir.dt.bfloat16
    Exp = mybir.ActivationFunctionType.Exp

    assert hpc == 2 and hdim == 64, "layout hardcoded for 2 heads x 64 dim"
    assert seq % 512 == 0
    nt = seq // 128   # number of 128-row seq tiles
    ng = seq // 512   # number of 512-wide q groups

    nc = bass.Bass()
    q = nc.dram_tensor("q", [seq, hpc, hdim], f32, kind="ExternalInput").ap()
    k = nc.dram_tensor("k", [seq, hpc, hdim], f32, kind="ExternalInput").ap()
    v = nc.dram_tensor("v", [seq, hpc, hdim], f32, kind="ExternalInput").ap()
    o = nc.dram_tensor("o", [seq, hpc * hdim], f32, kind="ExternalOutput").ap()

    with tile.TileContext(nc) as tc:
        with (
            tc.tile_pool(name="persist", bufs=1) as persist,
            tc.tile_pool(name="pexp", bufs=3) as pexp_pool,
            tc.tile_pool(name="outp", bufs=2) as out_pool,
            tc.tile_pool(name="small", bufs=4) as small_pool,
        ):
            # ---- persistent SBUF tensors ----------------------------------
            qT = persist.tile([128, seq], bf16, tag="qT")
            kT = persist.tile([128, seq], bf16, tag="kT")
            vplus = [
                persist.tile([128, nt * (hdim + 1)], bf16, tag=f"vplus{h}", name=f"vplus{h}")
                for h in range(hpc)
            ]
            # One packed multiplicative causal mask for the diagonal group:
            # local triangles (keep iff qi_local >= kj) at the 4 offsets,
            # zeros in the two pad gaps.
            maskp = persist.tile([128, 1536], bf16, tag="maskp")

            def build_masks():
                nc.vector.memset(maskp, 1.0)
                for t in range(4):
                    off, w = DIAG_OFF[t], DIAG_W[t]
                    nc.gpsimd.affine_select(
                        out=maskp[:, off : off + w],
                        in_=maskp[:, off : off + w],
                        compare_op=mybir.AluOpType.is_ge,
                        fill=0.0,
                        base=0,
                        pattern=[[1, w]],
                        channel_multiplier=-1,
                    )
                nc.vector.memset(maskp[:, 896:1024], 0.0)
                nc.vector.memset(maskp[:, 1408:1536], 0.0)

            # ---- V load: HWDGE fp32 (stays off the serial SWDGE cast
            # queue), DVE copy-casts into the 65-col vplus slots, ones
            # column memset separately (strided, cheap).
            vstage = persist.tile([128, hpc * nt * hdim], f32, tag="vstage")

            def load_v():
                nc.sync.dma_start(
                    out=vstage.rearrange(
                        "p (h t d) -> p h t d", h=hpc, d=hdim
                    ),
                    in_=v.rearrange("(t p) h d -> p h t d", p=128),
                )
                for h in range(hpc):
                    vp3 = vplus[h].rearrange("p (t x) -> p t x", x=hdim + 1)
                    nc.vector.memset(vp3[:, :, hdim : hdim + 1], 1.0)
                    nc.vector.tensor_copy(
                        vp3[:, :, 0:hdim],
                        vstage.rearrange(
                            "p (h t d) -> p h t d", h=hpc, d=hdim
                        )[:, h],
                    )

            # ---- Q/K load + transpose --------------------------------------
            # SWDGE cast DMAs are serial (single ucode queue), so order them
            # by criticality: chunk0 (tiles 0-7 of k then q, gates mm1 of
            # G0/G1), then the remaining 24 tiles as one big cast per tensor,
            # then V. Chunk0 is transposed on the PE (idle at startup, low
            # latency); the rest via the DMA XBAR (no engine cost, overlaps
            # the main loop's early groups).
            chunk = min(8, nt)
            rest = nt - chunk
            identity = persist.tile([128, 128], bf16, tag="identity")
            from concourse.masks import make_identity

            make_identity(nc, identity[:])
            with (
                tc.tile_pool(name="ldstage", bufs=1) as ld_pool,
                tc.tile_pool(name="psum_tr", bufs=4, space="PSUM") as tr_pool,
            ):
                srcs = {"k": (k, kT), "q": (q, qT)}

                def cast_load(nm, t0, t1):
                    src, _ = srcs[nm]
                    src_r = src.rearrange("(t p) h d -> p t (h d)", p=128)
                    st = ld_pool.tile(
                        [128, (t1 - t0) * 128], bf16, tag=f"ld{nm}{t0}"
                    )
                    nc.gpsimd.dma_start(
                        out=st.rearrange("p (t x) -> p t x", x=128),
                        in_=src_r[:, t0:t1, :],
                    )
                    return st

                def xbar_tr(nm, st, t0, ntile):
                    _, dstT = srcs[nm]
                    nc.sync.dma_start_transpose(
                        out=dstT[
                            :, t0 * 128 : (t0 + ntile) * 128
                        ].rearrange("p (c j) -> p c j", j=128),
                        in_=st[:],
                    )

                # SWDGE chain is pure q/k casts: chunk0 (tiles 0-7) first,
                # then c1 (8-15, gates G2/G3), then c23. V rides HWDGE in
                # parallel; masks build on gpsimd/DVE engines.
                st0 = {nm: cast_load(nm, 0, chunk) for nm in ("k", "q")}
                load_v()
                build_masks()
                st1 = {nm: cast_load(nm, chunk, 2 * chunk) for nm in ("k", "q")}
                for nm in ("k", "q"):
                    xbar_tr(nm, st1[nm], chunk, chunk)
                st2 = {nm: cast_load(nm, 2 * chunk, nt) for nm in ("k", "q")}
                for nm in ("k", "q"):
                    xbar_tr(nm, st2[nm], 2 * chunk, nt - 2 * chunk)
                # chunk0: PE transpose + DVE copy into kT/qT, interleaved in
                # 4-tile pieces so mm1 of G0 (tiles 0-3) unblocks earliest.
                for piece in range(chunk // 4):
                    for nm in ("k", "q"):
                        _, dstT = srcs[nm]
                        st = st0[nm]
                        for tt in range(piece * 4, piece * 4 + 4):
                            ptr = tr_pool.tile(
                                [128, 128], bf16, tag="ptr", name="ptr"
                            )
                            nc.tensor.transpose(
                                ptr[:], st[:, tt * 128 : (tt + 1) * 128],
                                identity[:],
                            )
                            nc.vector.tensor_copy(
                                dstT[:, tt * 128 : (tt + 1) * 128], ptr[:]
                            )

            # ---- main loop -------------------------------------------------
            with (
                tc.tile_pool(name="psum_s", bufs=1, space="PSUM") as ps_pool,
                tc.tile_pool(name="psum_o", bufs=1, space="PSUM") as po_pool,
            ):
                _main_loop(
                    nc, mybir, ng, hdim, ps_pool, po_pool, pexp_pool,
                    out_pool, small_pool, qT, kT, vplus, maskp, o, hpc, Exp,
                )
    _split_multi_waits(nc)
    return nc


def _split_multi_waits(nc):
    """Walrus's codegen accepts at most one sync-wait per instruction on
    this toolchain. Hoist extra waits into standalone single-wait NoOps on
    the same engine queue (same semantics: the sequencer stalls in order)."""
    import concourse.mybir as mybir

    nsplit = 0
    for blk in nc.m.functions[0].blocks:
        newl = []
        for ins in blk.instructions:
            si = getattr(ins, "sync_info", None)
            if si is not None and si.on_wait and len(si.on_wait) > 1:
                waits = list(si.on_wait)
                for w in waits[:-1]:
                    newl.append(
                        mybir.InstNoOp(
                            name=f"{ins.name}-wsplit{nsplit}",
                            sync_info=mybir.SyncInfo(on_wait=[w], on_update=[]),
                            bass_nofuse=True,
                            engine=ins.engine,
                            ins=[],
                            outs=[],
                        )
                    )
                    nsplit += 1
                ins.sync_info = mybir.SyncInfo(
                    on_wait=[waits[-1]], on_update=list(si.on_update or [])
                )
            newl.append(ins)
        blk.instructions = newl
    return nsplit


def _main_loop(nc, mybir, ng, hdim, ps_pool, po_pool, pexp_pool,
               out_pool, small_pool, qT, kT, vplus, maskp, o, hpc, Exp):
    SCALE = 0.125
    f32 = mybir.dt.float32
    bf16 = mybir.dt.bfloat16

    def emit_mm2s(st, drain):
        """Deferred V-stationary P@V accumulation for one jgroup. When
        `drain` is set this is the last jgroup of its G: each head's O^T is
        drained right after that head's final mm2 so the copy/transpose/
        normalize chain overlaps the other head's matmuls."""
        G, blocks, po, pes, njs, _ = st
        for h in range(hpc):
            for (j, off, w) in blocks:
                q0 = 512 - w  # valid q columns [q0:512] (0 for below-diag)
                nc.tensor.matmul(
                    po[h][0:hdim + 1, q0:512],
                    lhsT=vplus[h][:, j * 65 : j * 65 + hdim + 1],
                    rhs=pes[h][:, off : off + w],
                    start=(j == 0),
                    stop=(j == njs - 1),
                    skip_group_check=True,
                )
            if drain:
                emit_drain_head(G, po, h)
        if drain:
            emit_drain_finish(G)

    def emit_drain_head(G, po, h):
        # O^T [65, 512] (PSUM fp32) -> bf16 SBUF, XBAR-transpose to [q, d].
        oT = out_pool.tile([128, 512], bf16, tag=f"oT{h}", name=f"oT{h}")
        nc.vector.tensor_copy(oT[0 : hdim + 1, :], po[h][:, :])
        oTr = drain_state[h] = out_pool.tile(
            [128, 512], bf16, tag=f"oTr{h}", name=f"oTr{h}"
        )
        nc.sync.dma_start_transpose(
            out=oTr.rearrange("p (c j) -> p c j", j=128), in_=oT[:]
        )

    def emit_drain_finish(G):
        # reciprocal of the ones rows, scale, interleave heads, DMA out.
        for cc in range(4):
            ob = out_pool.tile([128, hpc * hdim], f32, tag="ob", name="ob")
            for h in range(hpc):
                rec = small_pool.tile([128, 1], f32, tag="rec", name="rec")
                nc.vector.reciprocal(
                    rec,
                    drain_state[h][:, cc * 128 + hdim : cc * 128 + hdim + 1],
                )
                nc.vector.tensor_scalar_mul(
                    ob[:, h * hdim : (h + 1) * hdim],
                    drain_state[h][:, cc * 128 : cc * 128 + hdim],
                    rec,
                )
            blk = G * 4 + cc
            nc.sync.dma_start(out=o[blk * 128 : (blk + 1) * 128, :], in_=ob[:])

    drain_state = {}

    pending = []  # deferred mm2 states (depth 2: PE stays 2 jgroups behind)
    for G in range(ng):
        njs = 4 * G + 4  # causal: k blocks 0 .. 4G+3
        po = [
            po_pool.tile([hdim + 1, 512], f32, tag=f"po{h}", name=f"po{h}")
            for h in range(hpc)
        ]
        # jgroups: below-diagonal full-width groups of <=3, then the packed
        # diagonal group (4 blocks at DIAG_OFF/DIAG_W).
        jgroups = []
        for s in range(0, 4 * G, 3):
            js = list(range(s, min(s + 3, 4 * G)))
            jgroups.append([(j, 512 * i, 512) for i, j in enumerate(js)])
        jgroups.append(
            [(4 * G + t, DIAG_OFF[t], DIAG_W[t]) for t in range(4)]
        )
        for gi, blocks in enumerate(jgroups):
            is_diag = gi == len(jgroups) - 1
            width = 1536 if is_diag else 512 * len(blocks)
            ps = [
                ps_pool.tile([128, 1536], f32, tag=f"ps{h}", name=f"ps{h}")
                for h in range(hpc)
            ]
            # mm1: S^T blocks, heads interleaved so LDWEIGHTS of the next
            # matmul (other 64-row group) overlaps the current stream.
            for (j, off, w) in blocks:
                t = j - 4 * G
                for h in range(hpc):
                    nc.tensor.matmul(
                        ps[h][:, off : off + w],
                        lhsT=kT[h * 64 : (h + 1) * 64, j * 128 : (j + 1) * 128],
                        rhs=qT[
                            h * 64 : (h + 1) * 64,
                            G * 512 + (512 - w) : (G + 1) * 512,
                        ],
                        start=not (is_diag and t == 3),
                        stop=True,
                        skip_group_check=True,
                        tile_position=(h * 64, 0),
                    )
            ew = 1408 if is_diag else width  # [1408:1536] is never consumed
            pes = []
            for h in range(hpc):
                pe = pexp_pool.tile(
                    [128, 1536], bf16, tag=f"pexp{h}", name=f"pexp{h}"
                )
                nc.scalar.activation(
                    out=pe[:, 0:ew], in_=ps[h][:, 0:ew], func=Exp,
                    scale=SCALE,
                )
                if is_diag:
                    nc.vector.tensor_mul(
                        pe[:, 0:1408], pe[:, 0:1408], maskp[:, 0:1408]
                    )
                pes.append(pe)
            pending.append((G, blocks, po, pes, njs, is_diag))
            if len(pending) > 1:
                st = pending.pop(0)
                emit_mm2s(st, st[5])
    for st in pending:
        emit_mm2s(st, st[5])


def _ensure_ntff_hook():
    """The image's antenv package lacks axon_hooks; provide it so
    run_bass_kernel_spmd's trace path works (or degrades gracefully)."""
    import sys
    import types

    try:
        import antenv.axon_hooks  # noqa: F401

        return
    except ImportError:
        pass
    mod = types.ModuleType("antenv.axon_hooks")
    state = {"hook": None}
    mod.set_axon_ntff_profile_hook = lambda h: state.__setitem__("hook", h)
    mod.get_axon_ntff_profile_hook = lambda: state["hook"]
    try:
        from trn_agent_boot.trn_boot import _ntff_profile_via_ctypes

        state["hook"] = _ntff_profile_via_ctypes("/opt/axon/libaxon_pjrt.so")
    except Exception:
        state["hook"] = None
    sys.modules["antenv.axon_hooks"] = mod


def kernel(q, k, v):
    """Full-input entry point: q, k, v [4096, 16, 64] fp32 -> [4096, 1024]."""
    import sys

    if "/opt/trn_rl_repo" not in sys.path:
        sys.path.insert(0, "/opt/trn_rl_repo")
    _ensure_ntff_hook()
    from concourse.bass_utils import run_bass_kernel_spmd

    q = np.asarray(q, dtype=np.float32)
    k = np.asarray(k, dtype=np.float32)
    v = np.asarray(v, dtype=np.float32)
    seq, nhead, hdim = q.shape

    if "nc" not in _NC_CACHE:
        _NC_CACHE["nc"] = build_attention_nc(seq=seq, hpc=HPC, hdim=hdim)
    nc = _NC_CACHE["nc"]

    in_maps = []
    for c in range(NCORES):
        hs = slice(c * HPC, (c + 1) * HPC)
        in_maps.append(
            {
                "q": np.ascontiguousarray(q[:, hs, :]),
                "k": np.ascontiguousarray(k[:, hs, :]),
                "v": np.ascontiguousarray(v[:, hs, :]),
            }
        )
    res = run_bass_kernel_spmd(nc, in_maps, core_ids=list(range(NCORES)))
    LAST_RESULT["exec_time_ns"] = res.exec_time_ns
    try:
        iat = res.instructions_and_trace
        LAST_RESULT["trace_path"] = iat[1] if iat else None
    except Exception:
        LAST_RESULT["trace_path"] = None
    outs = [res.results[c]["o"] for c in range(NCORES)]
    return np.concatenate(outs, axis=1)


# revision 18
# speedup vs baseline: 1.1461x; 1.0213x over previous
"""Trainium2 Bass kernel for multi-head causal attention.

Problem: q, k, v of shape [4096, 16, 64] (seq, heads, head_dim) fp32.
  out = softmax(causal(q @ k^T / 8)) @ v, reshaped to [4096, 1024].

Sharding: heads are split across 8 NeuronCores (2 heads per core).
Each core runs the same SPMD Bass program on its own 2 heads; the host
concatenates the per-core [4096, 128] outputs along the feature dim.

Per-core algorithm (flash-attention style, S^T orientation):
  - Load Q, K as bf16 (SWDGE cast DMA), transpose via the DMA XBAR
    (dma_start_transpose) into qT/kT [128=(h,d), 4096].
  - Load V per head into vplus [128, 32*65] bf16: each 128-row k-block
    gets 64 V columns plus a ones column (fused softmax denominator).
  - For each 512-wide q group G, per head h:
      mm1:  S^T[kj, qi] = kT_j^T.T @ qT_G into ps_h PSUM, 3 k-blocks per
            slot; the 4 diagonal blocks are PACKED (widths 512/384/256/128
            at offsets 0/512/1024/1280) so exp skips masked columns.
      exp:  one ScalarE activation per (group, head): Exp, scale=1/8,
            PSUM -> SBUF bf16.
      mask: diagonal group multiplied by one packed 0/1 causal mask (DVE).
      mm2:  V-STATIONARY: O^T[d|ones, qi] += vplus_j^T @ expS^T_j
            accumulated over ALL j in PSUM po_h [65, 512]. One N=512
            matmul per (j, head) instead of four M=128 ones.
  - Drain per G: DVE-copy po_h to bf16, DMA-XBAR transpose to [q, d]
    layout, reciprocal of the ones row, row-scale, DMA out.

mm2s are emitted one jgroup behind mm1/exp (software pipelining) so the
in-order PE queue always has ready work while ACT computes exp.

No distributed primitives are needed: sharding is purely host-side.
"""

import numpy as np

SEQ = 4096
NHEAD = 16
HDIM = 64
NCORES = 8
HPC = NHEAD // NCORES  # heads per core = 2
SCALE = 0.125

_NC_CACHE = {}
LAST_RESULT = {}

# Packed diagonal-group layout: per diag sub-block t, (psum offset, width).
DIAG_OFF = [0, 512, 1024, 1280]
DIAG_W = [512, 384, 256, 128]


def build_attention_nc(seq=SEQ, hpc=HPC, hdim=HDIM):
    """Build the SPMD Bass program for one core handling `hpc` heads."""
    import concourse.bass as bass
    import concourse.mybir as mybir
    import concourse.tile as tile

    f32 = mybir.dt.float32
    bf16 = mybir.dt.bfloat16
    Exp = mybir.ActivationFunctionType.Exp

    assert hpc == 2 and hdim == 64, "layout hardcoded for 2 heads x 64 dim"
    assert seq % 512 == 0
    nt = seq // 128   # number of 128-row seq tiles
    ng = seq // 512   # number of 512-wide q groups

    nc = bass.Bass()
    q = nc.dram_tensor("q", [seq, hpc, hdim], f32, kind="ExternalInput").ap()
    k = nc.dram_tensor("k", [seq, hpc, hdim], f32, kind="ExternalInput").ap()
    v = nc.dram_tensor("v", [seq, hpc, hdim], f32, kind="ExternalInput").ap()
    o = nc.dram_tensor("o", [seq, hpc * hdim], f32, kind="ExternalOutput").ap()

    with tile.TileContext(nc) as tc:
        with (
            tc.tile_pool(name="persist", bufs=1) as persist,
            tc.tile_pool(name="pexp", bufs=3) as pexp_pool,
            tc.tile_pool(name="outp", bufs=2) as out_pool,
            tc.tile_pool(name="small", bufs=4) as small_pool,
        ):
            # ---- persistent SBUF tensors ----------------------------------
            qT = persist.tile([128, seq], bf16, tag="qT")
            kT = persist.tile([128, seq], bf16, tag="kT")
            vplus = [
                persist.tile([128, nt * (hdim + 1)], bf16, tag=f"vplus{h}", name=f"vplus{h}")
                for h in range(hpc)
            ]
            # One packed multiplicative causal mask for the diagonal group:
            # local triangles (keep iff qi_local >= kj) at the 4 offsets,
            # zeros in the two pad gaps.
            maskp = persist.tile([128, 1536], bf16, tag="maskp")

            def build_masks():
                nc.vector.memset(maskp, 1.0)
                for t in range(4):
                    off, w = DIAG_OFF[t], DIAG_W[t]
                    nc.gpsimd.affine_select(
                        out=maskp[:, off : off + w],
                        in_=maskp[:, off : off + w],
                        compare_op=mybir.AluOpType.is_ge,
                        fill=0.0,
                        base=0,
                        pattern=[[1, w]],
                        channel_multiplier=-1,
                    )
                nc.vector.memset(maskp[:, 896:1024], 0.0)
                nc.vector.memset(maskp[:, 1408:1536], 0.0)

            # ---- V load: HWDGE fp32 (stays off the serial SWDGE cast
            # queue), DVE copy-casts into the 65-col vplus slots, ones
            # column memset separately (strided, cheap).
            vstage = persist.tile([128, hpc * nt * hdim], f32, tag="vstage")

            def load_v():
                # ACT's HWDGE queue: keeps SP's queue free for the XBAR
                # transposes of the staging chunks (SP is FIFO per queue).
                vst4 = vstage.rearrange("p (h t d) -> p h t d", h=hpc, d=hdim)
                vsrc = v.rearrange("(t p) h d -> p h t d", p=128)
                for h in range(hpc):
                    nc.scalar.dma_start(out=vst4[:, h], in_=vsrc[:, h])
                for h in range(hpc):
                    vp3 = vplus[h].rearrange("p (t x) -> p t x", x=hdim + 1)
                    nc.vector.memset(vp3[:, :, hdim : hdim + 1], 1.0)
                    nc.vector.tensor_copy(
                        vp3[:, :, 0:hdim],
                        vstage.rearrange(
                            "p (h t d) -> p h t d", h=hpc, d=hdim
                        )[:, h],
                    )

            # ---- Q/K load + transpose --------------------------------------
            # SWDGE cast DMAs are serial (single ucode queue), so order them
            # by criticality: chunk0 (tiles 0-7 of k then q, gates mm1 of
            # G0/G1), then the remaining 24 tiles as one big cast per tensor,
            # then V. Chunk0 is transposed on the PE (idle at startup, low
            # latency); the rest via the DMA XBAR (no engine cost, overlaps
            # the main loop's early groups).
            chunk = min(8, nt)
            rest = nt - chunk
            identity = persist.tile([128, 128], bf16, tag="identity")
            from concourse.masks import make_identity

            make_identity(nc, identity[:])
            with (
                tc.tile_pool(name="ldstage", bufs=1) as ld_pool,
                tc.tile_pool(name="psum_tr", bufs=4, space="PSUM") as tr_pool,
            ):
                srcs = {"k": (k, kT), "q": (q, qT)}

                def cast_load(nm, t0, t1):
                    src, _ = srcs[nm]
                    src_r = src.rearrange("(t p) h d -> p t (h d)", p=128)
                    st = ld_pool.tile(
                        [128, (t1 - t0) * 128], bf16, tag=f"ld{nm}{t0}"
                    )
                    nc.gpsimd.dma_start(
                        out=st.rearrange("p (t x) -> p t x", x=128),
                        in_=src_r[:, t0:t1, :],
                    )
                    return st

                def xbar_tr(nm, st, t0, ntile):
                    _, dstT = srcs[nm]
                    nc.sync.dma_start_transpose(
                        out=dstT[
                            :, t0 * 128 : (t0 + ntile) * 128
                        ].rearrange("p (c j) -> p c j", j=128),
                        in_=st[:],
                    )

                # SWDGE chain is pure q/k casts: chunk0 (tiles 0-7) first,
                # then c1 (8-15, gates G2/G3), then c23. V rides HWDGE in
                # parallel; masks build on gpsimd/DVE engines.
                st0 = {nm: cast_load(nm, 0, chunk) for nm in ("k", "q")}
                load_v()
                build_masks()
                st1 = {nm: cast_load(nm, chunk, 2 * chunk) for nm in ("k", "q")}
                for nm in ("k", "q"):
                    xbar_tr(nm, st1[nm], chunk, chunk)
                st2 = {nm: cast_load(nm, 2 * chunk, nt) for nm in ("k", "q")}
                for nm in ("k", "q"):
                    xbar_tr(nm, st2[nm], 2 * chunk, nt - 2 * chunk)
                # chunk0: PE transpose + DVE copy into kT/qT, interleaved in
                # 4-tile pieces so mm1 of G0 (tiles 0-3) unblocks earliest.
                for piece in range(chunk // 4):
                    for nm in ("k", "q"):
                        _, dstT = srcs[nm]
                        st = st0[nm]
                        for tt in range(piece * 4, piece * 4 + 4):
                            ptr = tr_pool.tile(
                                [128, 128], bf16, tag="ptr", name="ptr"
                            )
                            nc.tensor.transpose(
                                ptr[:], st[:, tt * 128 : (tt + 1) * 128],
                                identity[:],
                            )
                            nc.vector.tensor_copy(
                                dstT[:, tt * 128 : (tt + 1) * 128], ptr[:]
                            )

            # ---- main loop -------------------------------------------------
            with (
                tc.tile_pool(name="psum_s", bufs=1, space="PSUM") as ps_pool,
                tc.tile_pool(name="psum_o", bufs=1, space="PSUM") as po_pool,
            ):
                _main_loop(
                    nc, mybir, ng, hdim, ps_pool, po_pool, pexp_pool,
                    out_pool, small_pool, qT, kT, vplus, maskp, o, hpc, Exp,
                )
    _split_multi_waits(nc)
    return nc


def _split_multi_waits(nc):
    """Walrus's codegen accepts at most one sync-wait per instruction on
    this toolchain. Hoist extra waits into standalone single-wait NoOps on
    the same engine queue (same semantics: the sequencer stalls in order)."""
    import concourse.mybir as mybir

    nsplit = 0
    for blk in nc.m.functions[0].blocks:
        newl = []
        for ins in blk.instructions:
            si = getattr(ins, "sync_info", None)
            if si is not None and si.on_wait and len(si.on_wait) > 1:
                waits = list(si.on_wait)
                for w in waits[:-1]:
                    newl.append(
                        mybir.InstNoOp(
                            name=f"{ins.name}-wsplit{nsplit}",
                            sync_info=mybir.SyncInfo(on_wait=[w], on_update=[]),
                            bass_nofuse=True,
                            engine=ins.engine,
                            ins=[],
                            outs=[],
                        )
                    )
                    nsplit += 1
                ins.sync_info = mybir.SyncInfo(
                    on_wait=[waits[-1]], on_update=list(si.on_update or [])
                )
            newl.append(ins)
        blk.instructions = newl
    return nsplit


def _main_loop(nc, mybir, ng, hdim, ps_pool, po_pool, pexp_pool,
               out_pool, small_pool, qT, kT, vplus, maskp, o, hpc, Exp):
    SCALE = 0.125
    f32 = mybir.dt.float32
    bf16 = mybir.dt.bfloat16

    def emit_mm2s(st, drain):
        """Deferred V-stationary P@V accumulation for one jgroup. When
        `drain` is set this is the last jgroup of its G: each head's O^T is
        drained right after that head's final mm2 so the copy/transpose/
        normalize chain overlaps the other head's matmuls."""
        G, blocks, po, pes, njs, _ = st
        for h in range(hpc):
            for (j, off, w) in blocks:
                q0 = 512 - w  # valid q columns [q0:512] (0 for below-diag)
                nc.tensor.matmul(
                    po[h][0:hdim + 1, q0:512],
                    lhsT=vplus[h][:, j * 65 : j * 65 + hdim + 1],
                    rhs=pes[h][:, off : off + w],
                    start=(j == 0),
                    stop=(j == njs - 1),
                    skip_group_check=True,
                )
            if drain:
                emit_drain_head(G, po, h)
        if drain:
            emit_drain_finish(G)

    def emit_drain_head(G, po, h):
        # O^T [65, 512] (PSUM fp32) -> bf16 SBUF, XBAR-transpose to [q, d].
        oT = out_pool.tile([128, 512], bf16, tag=f"oT{h}", name=f"oT{h}")
        nc.vector.tensor_copy(oT[0 : hdim + 1, :], po[h][:, :])
        oTr = drain_state[h] = out_pool.tile(
            [128, 512], bf16, tag=f"oTr{h}", name=f"oTr{h}"
        )
        nc.sync.dma_start_transpose(
            out=oTr.rearrange("p (c j) -> p c j", j=128), in_=oT[:]
        )

    def emit_drain_finish(G):
        # reciprocal of the ones rows, scale, interleave heads, DMA out.
        for cc in range(4):
            ob = out_pool.tile([128, hpc * hdim], f32, tag="ob", name="ob")
            for h in range(hpc):
                rec = small_pool.tile([128, 1], f32, tag="rec", name="rec")
                nc.vector.reciprocal(
                    rec,
                    drain_state[h][:, cc * 128 + hdim : cc * 128 + hdim + 1],
                )
                nc.vector.tensor_scalar_mul(
                    ob[:, h * hdim : (h + 1) * hdim],
                    drain_state[h][:, cc * 128 : cc * 128 + hdim],
                    rec,
                )
            blk = G * 4 + cc
            nc.sync.dma_start(out=o[blk * 128 : (blk + 1) * 128, :], in_=ob[:])

    drain_state = {}

    pending = []  # deferred mm2 states (depth 2: PE stays 2 jgroups behind)
    for G in range(ng):
        njs = 4 * G + 4  # causal: k blocks 0 .. 4G+3
        po = [
            po_pool.tile([hdim + 1, 512], f32, tag=f"po{h}", name=f"po{h}")
            for h in range(hpc)
        ]
        # jgroups: below-diagonal full-width groups of <=3, then the packed
        # diagonal group (4 blocks at DIAG_OFF/DIAG_W).
        jgroups = []
        for s in range(0, 4 * G, 3):
            js = list(range(s, min(s + 3, 4 * G)))
            jgroups.append([(j, 512 * i, 512) for i, j in enumerate(js)])
        jgroups.append(
            [(4 * G + t, DIAG_OFF[t], DIAG_W[t]) for t in range(4)]
        )
        for gi, blocks in enumerate(jgroups):
            is_diag = gi == len(jgroups) - 1
            width = 1536 if is_diag else 512 * len(blocks)
            ps = [
                ps_pool.tile([128, 1536], f32, tag=f"ps{h}", name=f"ps{h}")
                for h in range(hpc)
            ]
            # mm1: S^T blocks, heads interleaved so LDWEIGHTS of the next
            # matmul (other 64-row group) overlaps the current stream.
            for (j, off, w) in blocks:
                t = j - 4 * G
                for h in range(hpc):
                    nc.tensor.matmul(
                        ps[h][:, off : off + w],
                        lhsT=kT[h * 64 : (h + 1) * 64, j * 128 : (j + 1) * 128],
                        rhs=qT[
                            h * 64 : (h + 1) * 64,
                            G * 512 + (512 - w) : (G + 1) * 512,
                        ],
                        start=not (is_diag and t == 3),
                        stop=True,
                        skip_group_check=True,
                        tile_position=(h * 64, 0),
                    )
            ew = 1408 if is_diag else width  # [1408:1536] is never consumed
            pes = []
            for h in range(hpc):
                pe = pexp_pool.tile(
                    [128, 1536], bf16, tag=f"pexp{h}", name=f"pexp{h}"
                )
                nc.scalar.activation(
                    out=pe[:, 0:ew], in_=ps[h][:, 0:ew], func=Exp,
                    scale=SCALE,
                )
                if is_diag:
                    nc.vector.tensor_mul(
                        pe[:, 0:1408], pe[:, 0:1408], maskp[:, 0:1408]
                    )
                pes.append(pe)
            pending.append((G, blocks, po, pes, njs, is_diag))
            if len(pending) > 1:
                st = pending.pop(0)
                emit_mm2s(st, st[5])
    for st in pending:
        emit_mm2s(st, st[5])


def _ensure_ntff_hook():
    """The image's antenv package lacks axon_hooks; provide it so
    run_bass_kernel_spmd's trace path works (or degrades gracefully)."""
    import sys
    import types

    try:
        import antenv.axon_hooks  # noqa: F401

        return
    except ImportError:
        pass
    mod = types.ModuleType("antenv.axon_hooks")
    state = {"hook": None}
    mod.set_axon_ntff_profile_hook = lambda h: state.__setitem__("hook", h)
    mod.get_axon_ntff_profile_hook = lambda: state["hook"]
    try:
        from trn_agent_boot.trn_boot import _ntff_profile_via_ctypes

        state["hook"] = _ntff_profile_via_ctypes("/opt/axon/libaxon_pjrt.so")
    except Exception:
        state["hook"] = None
    sys.modules["antenv.axon_hooks"] = mod


def kernel(q, k, v):
    """Full-input entry point: q, k, v [4096, 16, 64] fp32 -> [4096, 1024]."""
    import sys

    if "/opt/trn_rl_repo" not in sys.path:
        sys.path.insert(0, "/opt/trn_rl_repo")
    _ensure_ntff_hook()
    from concourse.bass_utils import run_bass_kernel_spmd

    q = np.asarray(q, dtype=np.float32)
    k = np.asarray(k, dtype=np.float32)
    v = np.asarray(v, dtype=np.float32)
    seq, nhead, hdim = q.shape

    if "nc" not in _NC_CACHE:
        _NC_CACHE["nc"] = build_attention_nc(seq=seq, hpc=HPC, hdim=hdim)
    nc = _NC_CACHE["nc"]

    in_maps = []
    for c in range(NCORES):
        hs = slice(c * HPC, (c + 1) * HPC)
        in_maps.append(
            {
                "q": np.ascontiguousarray(q[:, hs, :]),
                "k": np.ascontiguousarray(k[:, hs, :]),
                "v": np.ascontiguousarray(v[:, hs, :]),
            }
        )
    res = run_bass_kernel_spmd(nc, in_maps, core_ids=list(range(NCORES)))
    LAST_RESULT["exec_time_ns"] = res.exec_time_ns
    try:
        iat = res.instructions_and_trace
        LAST_RESULT["trace_path"] = iat[1] if iat else None
    except Exception:
        LAST_RESULT["trace_path"] = None
    outs = [res.results[c]["o"] for c in range(NCORES)]
    return np.concatenate(outs, axis=1)
